# revision 4
# baseline (speedup 1.0000x reference)
"""Batched int8-valued GEMM with dequant epilogue on 8 Trainium2 NeuronCores.

Problem: a[64,1024,128] i32 (vals 0..126), b[64,1024,128] i32 (vals 0..126),
alpha[1] f32.  out[bt,m,n] = fp16(alpha * sum_k a[bt,m,k]*b[bt,n,k]).

Sharding: pure batch-parallel — 8 batches per core, no communication.

Design (per core; HBM-bound: 16.78 MB stores + 2.1 MB int8 loads = 18.9 MB
@ ~360-415 GB/s/core; measured fixed tail ~8.6 us after the last DMA):
  - Host prep: int32 inputs narrowed to int8 (values 0..126, exact) and
    pre-transposed to K-major [k, ib, t, p] (a; row m = 8p+t) / [k, ib, n]
    (b).  K lands on partitions with no on-chip transposes.
  - Loads are plain int8 HWDGE — NO SWDGE anywhere: the SWDGE descriptor
    rings sit on SBUF partitions whose AXI ports serve SDMA engines 7/15,
    and with SWDGE cast-loads two of five runs had engine 15 running ~20%
    slow, then single-handedly draining a 10 us backlog tail.  b0/a0 ride
    the sync ring right behind alpha (land ~3 us); batches 1-7 go as four
    384/512 KiB chunks on the scalar ring (v3 lesson: 14 per-batch load
    dispatches on the ACT ring hit ring-slot backpressure and blocked the
    ACT sequencer ~10 us, starving the epilogue).
  - int8 -> bf16 casts on-chip: batch 0 on DVE (b0) + ACT (a0) for the
    fill, batches 1-7 on the otherwise-idle GpSimd (~1.7 us per [128,1024]
    tile, 2 casts/batch vs the ~6.2 us/batch cadence; fp8 operands were
    tried instead and the PE runs fp8 matmuls at ~380 ns/512-col vs bf16's
    216 ns warm — fp8 put the PE on the critical path).
  - Matmuls: per m-tile t, lhsT = aT[:, ib,t,:] [128k,128p] bf16, rhs =
    bT [128k,512n] bf16 x2 -> [128,1024] f32 PSUM (2 banks).  16 MM/batch
    ~3.5 us warm vs ~6.2 us/batch cadence.
  - alpha folded into the epilogue: ACT activation(Copy, scale=alpha_bc) /
    DVE tensor_scalar_mul — same cost as a plain copy, f32->fp16, exact
    (bf16 holds 0..126 exactly; products accumulate exactly in f32).
    alpha_bc [128,1] made once via ones-row PE broadcast.
  - Epilogue: 4 ACT / 4 DVE [128,1024] copies per batch.
  - Stores: m = 8p+t row interleave -> each [128, 4x1024] fp16 half is one
    1 MiB HWDGE store with 8 KiB/partition contiguous runs on the sync
    ring.  First batch half-0 and last batch half-1 go as per-m-tile
    256 KiB quarters (start the HBM write stream at ~7 us; shorten the
    final drain), tail quarters alternating the two HWDGE rings.  outp
    bufs=8 covers store drain + ~2 us HBM completion latency so the
    epilogue->psm->PE chain never backs up.
"""

import numpy as np

B, M, N, K = 64, 1024, 1024, 128
NCORES = 8
BPC = B // NCORES  # batches per core
TM = M // 128  # m tiles per batch (8)

_CACHE = {}


def _build_module():
    from contextlib import ExitStack

    import concourse.tile as tile
    from concourse import bacc, mybir
    from concourse.bass import ds

    fp16 = mybir.dt.float16
    bf16 = mybir.dt.bfloat16
    f32 = mybir.dt.float32
    i8 = mybir.dt.int8

    nc = bacc.Bacc("TRN2", debug=False, enable_asserts=False)
    # K-major int8 inputs (host-side exact narrowing + transpose):
    #   aT[k, ib*1024 + t*128 + p] = a[ib, 8p+t, k]
    #   bT[k, ib*1024 + n]         = b[ib, n, k]
    a_d = nc.dram_tensor("a", [128, BPC * M], i8, kind="ExternalInput")
    b_d = nc.dram_tensor("b", [128, BPC * N], i8, kind="ExternalInput")
    al_d = nc.dram_tensor("alpha", [1], f32, kind="ExternalInput")
    o_d = nc.dram_tensor("out", [BPC, M, N], fp16, kind="ExternalOutput")

    with ExitStack() as ctx:
        tc = ctx.enter_context(tile.TileContext(nc))
        const = ctx.enter_context(tc.tile_pool(name="const", bufs=1))
        inp = ctx.enter_context(tc.tile_pool(name="inp", bufs=1))
        outp = ctx.enter_context(tc.tile_pool(name="outp", bufs=8))
        pst = ctx.enter_context(tc.tile_pool(name="pst", bufs=1, space="PSUM"))
        psm = ctx.enter_context(tc.tile_pool(name="psm", bufs=3, space="PSUM"))

        a_raw = inp.tile([128, BPC * M], i8, tag="a_raw")
        b_raw = inp.tile([128, BPC * N], i8, tag="b_raw")
        a_bf = inp.tile([128, BPC * M], bf16, tag="a_bf")
        b_bf = inp.tile([128, BPC * N], bf16, tag="b_bf")

        # alpha + batch-0 raw loads on the sync ring (stores queue after)
        alpha_1 = const.tile([1, 1], f32)
        nc.sync.dma_start(alpha_1[:], al_d.ap().rearrange("(a x) -> a x", a=1))
        nc.sync.dma_start(b_raw[:, ds(0, 1024)], b_d.ap()[:, ds(0, 1024)])
        nc.sync.dma_start(a_raw[:, ds(0, 1024)], a_d.ap()[:, ds(0, 1024)])
        # batches 1-7 as four chunked loads on the scalar ring
        nc.scalar.dma_start(b_raw[:, ds(1024, 3072)], b_d.ap()[:, ds(1024, 3072)])
        nc.scalar.dma_start(a_raw[:, ds(1024, 3072)], a_d.ap()[:, ds(1024, 3072)])
        nc.scalar.dma_start(b_raw[:, ds(4096, 4096)], b_d.ap()[:, ds(4096, 4096)])
        nc.scalar.dma_start(a_raw[:, ds(4096, 4096)], a_d.ap()[:, ds(4096, 4096)])

        # alpha broadcast to [128,1] via PE: ones_row.T @ alpha (contraction=1)
        ones_row = const.tile([1, 128], f32)
        nc.vector.memset(ones_row[:], 1.0)
        alpha_ps = pst.tile([128, 1], f32, tag="aps")
        nc.tensor.matmul(alpha_ps[:], ones_row[:], alpha_1[:], start=True, stop=True)
        alpha_bc = const.tile([128, 1], f32)
        nc.vector.tensor_copy(alpha_bc[:], alpha_ps[:])

        # batch-0 casts on DVE/ACT (fill path); batches 1-7 on GpSimd
        nc.vector.tensor_copy(b_bf[:, ds(0, 1024)], b_raw[:, ds(0, 1024)])
        nc.scalar.copy(a_bf[:, ds(0, 1024)], a_raw[:, ds(0, 1024)])
        for ib in range(1, BPC):
            nc.gpsimd.tensor_copy(
                b_bf[:, ds(ib * 1024, 1024)], b_raw[:, ds(ib * 1024, 1024)]
            )
            nc.gpsimd.tensor_copy(
                a_bf[:, ds(ib * 1024, 1024)], a_raw[:, ds(ib * 1024, 1024)]
            )

        for ib in range(BPC):
            aT = a_bf[:, ds(ib * 1024, 1024)]
            bT = b_bf[:, ds(ib * 1024, 1024)]
            for half in range(2):
                out_sb = outp.tile([128, 4 * N], fp16, tag="out_sb")
                for tq in range(4):
                    t = 4 * half + tq
                    ps = psm.tile([128, 1024], f32)
                    for nh in range(2):
                        nc.tensor.matmul(
                            ps[:, ds(nh * 512, 512)],
                            aT[:, ds(t * 128, 128)],
                            bT[:, ds(nh * 512, 512)],
                            start=True,
                            stop=True,
                        )
                    o_slice = out_sb[:, ds(tq * N, N)]
                    # epilogue = dequant: out = fp16(alpha * acc), alternating
                    # ACT / DVE so both engines carry half the copy stream
                    if t % 2 == 0:
                        nc.scalar.activation(
                            o_slice,
                            ps[:],
                            mybir.ActivationFunctionType.Copy,
                            scale=alpha_bc[:],
                        )
                    else:
                        nc.vector.tensor_scalar_mul(o_slice, ps[:], alpha_bc[:])

                    # first half-batch + last half-batch: 256 KiB per-m-tile
                    # quarter stores (start the HBM write stream early / end
                    # drain on both HWDGE rings)
                    if (ib, half) == (0, 0):
                        nc.sync.dma_start(
                            o_d.ap()[ib].rearrange("(p t) n -> p t n", t=TM)[
                                :, t : t + 1, :
                            ],
                            o_slice.rearrange("p (t n) -> p t n", t=1),
                        )
                    elif (ib, half) == (BPC - 1, 1):
                        qeng = nc.sync if tq % 2 == 0 else nc.scalar
                        qeng.dma_start(
                            o_d.ap()[ib].rearrange("(p t) n -> p t n", t=TM)[
                                :, t : t + 1, :
                            ],
                            o_slice.rearrange("p (t n) -> p t n", t=1),
                        )

                # rows m = 8p+t, t in [4*half, 4*half+4): 8 KiB contiguous
                # per partition, 1 MiB per store on the sync HWDGE ring
                if (ib, half) not in ((0, 0), (BPC - 1, 1)):
                    nc.sync.dma_start(
                        o_d.ap()[ib].rearrange("(p t) n -> p t n", t=TM)[
                            :, 4 * half : 4 * half + 4, :
                        ],
                        out_sb[:].rearrange("p (t n) -> p t n", n=N),
                    )

    nc.compile()
    return nc


def _get_module():
    if "nc" not in _CACHE:
        _CACHE["nc"] = _build_module()
    return _CACHE["nc"]


def run(a, b, alpha, trace=False, **kw):
    from concourse.bass_utils import run_bass_kernel_spmd

    nc = _get_module()

    # values are 0..126: int8 narrowing is exact.  Host pre-transpose to
    # K-major so K sits on SBUF partitions with no on-chip transposes.
    a = np.ascontiguousarray(a).astype(np.int8)
    b = np.ascontiguousarray(b).astype(np.int8)
    # aT[c, k, ib, t, p] = a[c, ib, m=8p+t, k]
    a = a.reshape(NCORES, BPC, 128, TM, K).transpose(0, 4, 1, 3, 2)
    a = np.ascontiguousarray(a.reshape(NCORES, K, BPC * M))
    # bT[c, k, ib, n] = b[c, ib, n, k]
    b = b.reshape(NCORES, BPC, N, K).transpose(0, 3, 1, 2)
    b = np.ascontiguousarray(b.reshape(NCORES, K, BPC * N))
    alpha = np.ascontiguousarray(alpha, dtype=np.float32)
    in_maps = [{"a": a[i], "b": b[i], "alpha": alpha} for i in range(NCORES)]
    res = run_bass_kernel_spmd(
        nc, in_maps, core_ids=list(range(NCORES)), trace=trace, **kw
    )
    out = np.concatenate([r["out"] for r in res.results], axis=0)
    return out, res


def kernel(a, b, alpha):
    out, _ = run(a, b, alpha, trace=False)
    return out


# revision 7
# speedup vs baseline: 1.1985x; 1.1985x over previous
"""Batched int8-valued GEMM with dequant epilogue on 8 Trainium2 NeuronCores.

Problem: a[64,1024,128] i32 (vals 0..126), b[64,1024,128] i32 (vals 0..126),
alpha[1] f32.  out[bt,m,n] = fp16(alpha * sum_k a[bt,m,k]*b[bt,n,k]).

Sharding: pure batch-parallel — 8 batches per core, no communication.

Design (per core; HBM-bound: 16.78 MB stores + 2.1 MB int8 loads = 18.9 MB
@ ~365 GB/s/core measured; plus a fixed ~8.6 us post-DMA runtime tail):
  - Host prep: int32 inputs narrowed to int8 (values 0..126, exact) and
    pre-transposed to K-major [k, ib, t, p] (a; row m = 8p+t) / [k, ib, n]
    (b).  K lands on partitions with no on-chip transposes, no identity,
    no permuted epilogue APs.
  - SWDGE cast-DMAs int8 HBM -> bf16 SBUF.  Measured dead ends this
    replaces: on-chip int8->bf16 casts cost 4.3-4.7 us per [128,1024] on
    every engine (gpsimd/DVE/ACT); fp8 operands make the PE run matmuls
    at ~380 ns/512-col vs bf16's 216 ns warm.  The cast-DMA converts for
    free.  Batch-0 pieces go first in 512-col halves (first matmul gates
    on 1/4 of the batch), then per-batch loads with their own sems.
  - All stores are HWDGE — v2 put 3 MB of tail stores on the gpsimd SWDGE
    queue, and SWDGE descriptor-ring fetch traffic rides the SBUF AXI
    ports that serve SDMA engines 7/15 (2 of 5 v2 runs: engine 15 ~20%
    slow, draining a 10 us solo backlog at the end).  Loads are the only
    SWDGE users left.
  - Matmuls: per m-tile t, lhsT = aT[:, ib,t,:] [128k,128p] bf16, rhs =
    bT [128k,512n] bf16 x2 -> [128,1024] f32 PSUM (2 banks).  16 MM/batch
    ~3.5 us warm vs ~6 us/batch HBM cadence.  psm bufs=4 fills all 8 PSUM
    banks (alpha's broadcast borrows a psm slot at startup).
  - alpha folded into the epilogue: ACT activation(Copy, scale=alpha_bc) /
    DVE tensor_scalar_mul — same cost as a plain copy, f32->fp16, exact
    (bf16 holds 0..126 exactly; products accumulate exactly in f32; only
    the final fp16 round differs from the f64 reference).
  - Epilogue: 4 ACT / 4 DVE [128,1024] copies per batch (~4.8/5.1 us per
    ~6 us cadence).
  - Stores: m = 8p+t row interleave -> each [128, 4x1024] fp16 half is one
    1 MiB store with 8 KiB/partition contiguous runs on the sync ring.
    First batch half-0 and last batch half-1 go as per-m-tile 256 KiB
    quarters (start the HBM write stream at ~5.5 us; shorten the final
    drain), tail quarters alternating the sync/scalar HWDGE rings.  outp
    bufs=8 covers store drain + ~2 us HBM completion latency so the
    epilogue->psm->PE chain never backs up.
"""

import numpy as np

B, M, N, K = 64, 1024, 1024, 128
NCORES = 8
BPC = B // NCORES  # batches per core
TM = M // 128  # m tiles per batch (8)

_CACHE = {}


def _build_module():
    from contextlib import ExitStack

    import concourse.tile as tile
    from concourse import bacc, mybir
    from concourse.bass import ds

    fp16 = mybir.dt.float16
    bf16 = mybir.dt.bfloat16
    f32 = mybir.dt.float32
    i8 = mybir.dt.int8

    nc = bacc.Bacc("TRN2", debug=False, enable_asserts=False)
    # K-major int8 inputs (host-side exact narrowing + transpose):
    #   aT[k, ib*1024 + t*128 + p] = a[ib, 8p+t, k]
    #   bT[k, ib*1024 + n]         = b[ib, n, k]
    a_d = nc.dram_tensor("a", [128, BPC * M], i8, kind="ExternalInput")
    b_d = nc.dram_tensor("b", [128, BPC * N], i8, kind="ExternalInput")
    al_d = nc.dram_tensor("alpha", [1], f32, kind="ExternalInput")
    o_d = nc.dram_tensor("out", [BPC, M, N], fp16, kind="ExternalOutput")

    with ExitStack() as ctx:
        tc = ctx.enter_context(tile.TileContext(nc))
        const = ctx.enter_context(tc.tile_pool(name="const", bufs=1))
        inp = ctx.enter_context(tc.tile_pool(name="inp", bufs=1))
        outp = ctx.enter_context(tc.tile_pool(name="outp", bufs=8))
        pst = ctx.enter_context(tc.tile_pool(name="pst", bufs=1, space="PSUM"))
        psm = ctx.enter_context(tc.tile_pool(name="psm", bufs=3, space="PSUM"))

        a_all = inp.tile([128, BPC * M], bf16, tag="a_all")
        b_all = inp.tile([128, BPC * N], bf16, tag="b_all")

        # batch-0 cast-loads first, in 512-col pieces: the first matmul
        # gates on b0[:, :512] + a0[:, :512] only
        nc.gpsimd.dma_start(b_all[:, ds(0, 512)], b_d.ap()[:, ds(0, 512)])
        nc.gpsimd.dma_start(a_all[:, ds(0, 512)], a_d.ap()[:, ds(0, 512)])
        nc.gpsimd.dma_start(b_all[:, ds(512, 512)], b_d.ap()[:, ds(512, 512)])
        nc.gpsimd.dma_start(a_all[:, ds(512, 512)], a_d.ap()[:, ds(512, 512)])

        # alpha broadcast to [128,1] via PE: ones_row.T @ alpha (contraction=1)
        alpha_1 = const.tile([1, 1], f32)
        nc.sync.dma_start(alpha_1[:], al_d.ap().rearrange("(a x) -> a x", a=1))
        ones_row = const.tile([1, 128], f32)
        nc.vector.memset(ones_row[:], 1.0)
        alpha_ps = pst.tile([128, 1], f32, tag="aps")
        nc.tensor.matmul(alpha_ps[:], ones_row[:], alpha_1[:], start=True, stop=True)
        alpha_bc = const.tile([128, 1], f32)
        nc.vector.tensor_copy(alpha_bc[:], alpha_ps[:])

        # per-batch cast-loads for batches 1-7 (own completion sems so
        # batch k's matmuls gate only on its own 256 KiB)
        for ib in range(1, BPC):
            nc.gpsimd.dma_start(
                b_all[:, ds(ib * 1024, 1024)], b_d.ap()[:, ds(ib * 1024, 1024)]
            )
            nc.gpsimd.dma_start(
                a_all[:, ds(ib * 1024, 1024)], a_d.ap()[:, ds(ib * 1024, 1024)]
            )

        for ib in range(BPC):
            aT = a_all[:, ds(ib * 1024, 1024)]
            bT = b_all[:, ds(ib * 1024, 1024)]
            for half in range(2):
                out_sb = outp.tile([128, 4 * N], fp16, tag="out_sb")
                for tq in range(4):
                    t = 4 * half + tq
                    ps = psm.tile([128, 1024], f32)
                    for nh in range(2):
                        nc.tensor.matmul(
                            ps[:, ds(nh * 512, 512)],
                            aT[:, ds(t * 128, 128)],
                            bT[:, ds(nh * 512, 512)],
                            start=True,
                            stop=True,
                        )
                    o_slice = out_sb[:, ds(tq * N, N)]
                    # epilogue = dequant: out = fp16(alpha * acc), alternating
                    # ACT / DVE so both engines carry half the copy stream
                    if t % 2 == 0:
                        nc.scalar.activation(
                            o_slice,
                            ps[:],
                            mybir.ActivationFunctionType.Copy,
                            scale=alpha_bc[:],
                        )
                    else:
                        nc.vector.tensor_scalar_mul(o_slice, ps[:], alpha_bc[:])

                    # first half-batch + last half-batch: 256 KiB per-m-tile
                    # quarter stores (start the HBM write stream early / end
                    # drain on both HWDGE rings)
                    if (ib, half) == (0, 0):
                        nc.sync.dma_start(
                            o_d.ap()[ib].rearrange("(p t) n -> p t n", t=TM)[
                                :, t : t + 1, :
                            ],
                            o_slice.rearrange("p (t n) -> p t n", t=1),
                        )
                    elif (ib, half) == (BPC - 1, 1):
                        qeng = nc.sync if tq % 2 == 0 else nc.scalar
                        qeng.dma_start(
                            o_d.ap()[ib].rearrange("(p t) n -> p t n", t=TM)[
                                :, t : t + 1, :
                            ],
                            o_slice.rearrange("p (t n) -> p t n", t=1),
                        )

                # rows m = 8p+t, t in [4*half, 4*half+4): 8 KiB contiguous
                # per partition, 1 MiB per store on the sync HWDGE ring
                if (ib, half) not in ((0, 0), (BPC - 1, 1)):
                    nc.sync.dma_start(
                        o_d.ap()[ib].rearrange("(p t) n -> p t n", t=TM)[
                            :, 4 * half : 4 * half + 4, :
                        ],
                        out_sb[:].rearrange("p (t n) -> p t n", n=N),
                    )

    nc.compile()
    return nc


def _get_module():
    if "nc" not in _CACHE:
        _CACHE["nc"] = _build_module()
    return _CACHE["nc"]


def run(a, b, alpha, trace=False, **kw):
    from concourse.bass_utils import run_bass_kernel_spmd

    nc = _get_module()

    # values are 0..126: int8 narrowing is exact.  Host pre-transpose to
    # K-major so K sits on SBUF partitions with no on-chip transposes.
    a = np.ascontiguousarray(a).astype(np.int8)
    b = np.ascontiguousarray(b).astype(np.int8)
    # aT[c, k, ib, t, p] = a[c, ib, m=8p+t, k]
    a = a.reshape(NCORES, BPC, 128, TM, K).transpose(0, 4, 1, 3, 2)
    a = np.ascontiguousarray(a.reshape(NCORES, K, BPC * M))
    # bT[c, k, ib, n] = b[c, ib, n, k]
    b = b.reshape(NCORES, BPC, N, K).transpose(0, 3, 1, 2)
    b = np.ascontiguousarray(b.reshape(NCORES, K, BPC * N))
    alpha = np.ascontiguousarray(alpha, dtype=np.float32)
    in_maps = [{"a": a[i], "b": b[i], "alpha": alpha} for i in range(NCORES)]
    res = run_bass_kernel_spmd(
        nc, in_maps, core_ids=list(range(NCORES)), trace=trace, **kw
    )
    out = np.concatenate([r["out"] for r in res.results], axis=0)
    return out, res


def kernel(a, b, alpha):
    out, _ = run(a, b, alpha, trace=False)
    return out
